# revision 5
# baseline (speedup 1.0000x reference)
"""ChessNNUE Trainium2 kernel.

Strategy (data-parallel over batch, 8 NeuronCores):
  - Each core handles 512 of the 4096 batch rows; ft_w is replicated.
  - Host pre-transposes operands so every SBUF load is a straight
    contiguous DMA with the contraction dim on partitions:
        featT  [FEAT, 512]  per-core slice of features.T
        ft_wT  [FEAT, HID]
  - Feature-transformer matmul runs on the tensor engine in bf16
    (features are exactly representable 0/1; ft_w bf16 rounding gives
    ~1e-3 relative error), accumulating fp32 in PSUM over groups of
    K-tiles, with per-group evacuation-adds into SBUF fp32 accumulators.
  - f32->bf16 conversion happens inside the DMA (SWDGE cast) so no
    vector-engine cycles are spent on casts.
  - stm select/clip + the small MLP head run on-chip per core; outputs
    are [2, 512] (sigmoid row, raw row) per core, assembled on host.
"""

import numpy as np
from contextlib import ExitStack

import concourse.bass as bass
import concourse.tile as tile
from concourse import bacc, mybir
from concourse.bass_utils import run_bass_kernel_spmd

B, FEAT, HID = 4096, 40960, 1024
L1, L2 = 64, 32
NCORES = 8
BC = B // NCORES          # 512 batch rows per core
NHC = HID // 128          # 8 hid chunks of 128

F32 = mybir.dt.float32
BF16 = mybir.dt.bfloat16

_CACHE = {}


def _build(feat=FEAT, gk=16):
    """Build + compile the per-core Bass program. Returns nc."""
    kt = feat // 128          # number of K tiles
    assert kt % gk == 0
    ng = kt // gk             # number of K groups
    Alu = mybir.AluOpType
    Act = mybir.ActivationFunctionType

    nc = bacc.Bacc("TRN2", target_bir_lowering=False, debug=False,
                   num_devices=NCORES)

    fw = nc.dram_tensor("fw", (feat, BC), F32, kind="ExternalInput")
    fb = nc.dram_tensor("fb", (feat, BC), F32, kind="ExternalInput")
    wT = nc.dram_tensor("wT", (feat, HID), F32, kind="ExternalInput")
    ftb = nc.dram_tensor("ftb", (HID,), F32, kind="ExternalInput")
    stm = nc.dram_tensor("stm", (BC,), F32, kind="ExternalInput")
    l1w = nc.dram_tensor("l1w", (2 * HID, L1), F32, kind="ExternalInput")
    l1b = nc.dram_tensor("l1b", (L1,), F32, kind="ExternalInput")
    l2w = nc.dram_tensor("l2w", (L1, L2), F32, kind="ExternalInput")
    l2b = nc.dram_tensor("l2b", (L2,), F32, kind="ExternalInput")
    l3w = nc.dram_tensor("l3w", (L2, 1), F32, kind="ExternalInput")
    l3b = nc.dram_tensor("l3b", (1,), F32, kind="ExternalInput")
    out = nc.dram_tensor("out", (2, BC), F32, kind="ExternalOutput")

    with ExitStack() as ctx:
        tc = ctx.enter_context(tile.TileContext(nc))
        const = ctx.enter_context(tc.tile_pool(name="const", bufs=1))
        wpool = ctx.enter_context(tc.tile_pool(name="wpool", bufs=2))
        fwpool = ctx.enter_context(tc.tile_pool(name="fwpool", bufs=2))
        fbpool = ctx.enter_context(tc.tile_pool(name="fbpool", bufs=2))
        accpool = ctx.enter_context(tc.tile_pool(name="accpool", bufs=1))
        h1pool = ctx.enter_context(tc.tile_pool(name="h1pool", bufs=1))
        tmppool = ctx.enter_context(tc.tile_pool(name="tmppool", bufs=1))
        psum = ctx.enter_context(
            tc.tile_pool(name="psum", bufs=8, space="PSUM"))

        # ---------- constants ----------
        ftb_sb = const.tile([128, NHC], F32)
        nc.sync.dma_start(ftb_sb[:], ftb.ap().rearrange("(c p) -> p c", p=128))
        l1b_sb = const.tile([L1, 1], F32)
        nc.sync.dma_start(l1b_sb[:], l1b.ap())
        l2b_sb = const.tile([L2, 1], F32)
        nc.sync.dma_start(l2b_sb[:], l2b.ap())
        l3b_sb = const.tile([1, 1], F32)
        nc.sync.dma_start(l3b_sb[:], l3b.ap())

        l1w_sb = const.tile([128, (2 * HID) // 128, L1], BF16)
        nc.gpsimd.dma_start(l1w_sb[:],
                            l1w.ap().rearrange("(t p) m -> p t m", p=128))
        l2w_sb = const.tile([L1, L2], BF16)
        nc.gpsimd.dma_start(l2w_sb[:], l2w.ap())
        l3w_sb = const.tile([L2, 1], BF16)
        nc.gpsimd.dma_start(l3w_sb[:], l3w.ap())

        stm_bf = const.tile([1, BC], BF16)
        nc.gpsimd.dma_start(stm_bf[:], stm.ap())
        ones_bf = const.tile([1, 128], BF16)
        nc.vector.memset(ones_bf[:], 1.0)

        # broadcast stm across partitions: [128, BC] = ones[1,128].T @ stm[1,BC]
        ps_stm = psum.tile([128, BC], F32, tag="ps")
        nc.tensor.matmul(ps_stm[:], ones_bf[:], stm_bf[:],
                         start=True, stop=True)
        stmb_sb = const.tile([128, BC], F32)
        nc.vector.tensor_copy(stmb_sb[:], ps_stm[:])

        # persistent fp32 accumulators: [0..7] = white persp, [8..15] = black
        accs = [accpool.tile([128, BC], F32, tag=f"acc{i}", name=f"acc{i}")
                for i in range(16)]

        # ---------- feature transformer main loop ----------
        for g in range(ng):
            r0, r1 = g * gk * 128, (g + 1) * gk * 128
            wt = wpool.tile([128, gk, HID], BF16, tag="wt")
            nc.gpsimd.dma_start(
                wt[:], wT.ap()[r0:r1, :].rearrange("(t p) h -> p t h", p=128))
            fwt = fwpool.tile([128, gk, BC], BF16, tag="fwt")
            nc.gpsimd.dma_start(
                fwt[:], fw.ap()[r0:r1, :].rearrange("(t p) n -> p t n", p=128))
            fbt = fbpool.tile([128, gk, BC], BF16, tag="fbt")
            nc.gpsimd.dma_start(
                fbt[:], fb.ap()[r0:r1, :].rearrange("(t p) n -> p t n", p=128))

            for s, ftile in enumerate((fwt, fbt)):
                for c in range(NHC):
                    ps = psum.tile([128, BC], F32, tag="ps")
                    for t in range(gk):
                        nc.tensor.matmul(
                            ps[:],
                            wt[:, t, c * 128:(c + 1) * 128],
                            ftile[:, t, :],
                            start=(t == 0), stop=(t == gk - 1))
                    a = accs[s * NHC + c]
                    if g == 0:
                        # acc = psum + ft_b (fold the bias into first evac)
                        nc.vector.tensor_scalar(
                            a[:], ps[:], ftb_sb[:, c:c + 1], None, Alu.add)
                    else:
                        nc.vector.tensor_add(a[:], a[:], ps[:])

        # ---------- stm select + clip -> h1 (bf16) ----------
        h1s = [h1pool.tile([128, BC], BF16, tag=f"h1_{i}", name=f"h1_{i}")
               for i in range(16)]
        for c in range(NHC):
            w_, b_ = accs[c], accs[NHC + c]
            d = tmppool.tile([128, BC], F32, tag="d")
            nc.vector.tensor_sub(d[:], w_[:], b_[:])
            m = tmppool.tile([128, BC], F32, tag="m")
            nc.vector.tensor_mul(m[:], d[:], stmb_sb[:])
            topf = tmppool.tile([128, BC], F32, tag="topf")
            nc.vector.tensor_add(topf[:], b_[:], m[:])      # stm*w+(1-stm)*b
            botf = tmppool.tile([128, BC], F32, tag="botf")
            nc.vector.tensor_sub(botf[:], w_[:], m[:])      # stm*b+(1-stm)*w
            nc.vector.tensor_scalar(
                h1s[c][:], topf[:], 0.0, 1.0, Alu.max, Alu.min)
            nc.vector.tensor_scalar(
                h1s[NHC + c][:], botf[:], 0.0, 1.0, Alu.max, Alu.min)

        # ---------- head ----------
        ps1 = psum.tile([L1, BC], F32, tag="ps")
        for t in range(16):
            nc.tensor.matmul(ps1[:], l1w_sb[:, t, :], h1s[t][:],
                             start=(t == 0), stop=(t == 15))
        h2f = tmppool.tile([L1, BC], F32, tag="h2f")
        nc.vector.tensor_scalar(h2f[:], ps1[:], l1b_sb[:], 0.0, Alu.add, Alu.max)
        h2 = tmppool.tile([L1, BC], BF16, tag="h2")
        nc.vector.tensor_scalar(h2[:], h2f[:], 1.0, None, Alu.min)

        ps2 = psum.tile([L2, BC], F32, tag="ps")
        nc.tensor.matmul(ps2[:], l2w_sb[:], h2[:], start=True, stop=True)
        h3f = tmppool.tile([L2, BC], F32, tag="h3f")
        nc.vector.tensor_scalar(h3f[:], ps2[:], l2b_sb[:], 0.0, Alu.add, Alu.max)
        h3 = tmppool.tile([L2, BC], BF16, tag="h3")
        nc.vector.tensor_scalar(h3[:], h3f[:], 1.0, None, Alu.min)

        ps3 = psum.tile([1, BC], F32, tag="ps")
        nc.tensor.matmul(ps3[:], l3w_sb[:], h3[:], start=True, stop=True)

        sig_sb = const.tile([1, BC], F32)
        raw_sb = const.tile([1, BC], F32)
        nc.vector.tensor_scalar(raw_sb[:], ps3[:], l3b_sb[:], None, Alu.add)
        nc.scalar.activation(sig_sb[:], ps3[:], Act.Sigmoid, bias=l3b_sb[:])
        nc.sync.dma_start(out.ap()[0:1, :], sig_sb[:])
        nc.sync.dma_start(out.ap()[1:2, :], raw_sb[:])

    nc.compile()
    return nc


def _get_nc():
    if "nc" not in _CACHE:
        _CACHE["nc"] = _build()
    return _CACHE["nc"]


def _prep_in_maps(white_features, black_features, stm, ft_w, ft_b,
                  l1_w, l1_b, l2_w, l2_b, l3_w, l3_b):
    f32 = lambda a: np.ascontiguousarray(np.asarray(a, dtype=np.float32))
    white = np.asarray(white_features, dtype=np.float32)
    black = np.asarray(black_features, dtype=np.float32)
    stm = np.asarray(stm, dtype=np.float32).reshape(B)
    wT = f32(np.asarray(ft_w, dtype=np.float32).T)        # [FEAT, HID]
    l1wT = f32(np.asarray(l1_w, dtype=np.float32).T)      # [2048, 64]
    l2wT = f32(np.asarray(l2_w, dtype=np.float32).T)      # [64, 32]
    l3wT = f32(np.asarray(l3_w, dtype=np.float32).T)      # [32, 1]
    ftb = f32(ft_b)
    l1b, l2b, l3b = f32(l1_b), f32(l2_b), f32(l3_b)

    in_maps = []
    for c in range(NCORES):
        sl = slice(c * BC, (c + 1) * BC)
        in_maps.append(dict(
            fw=f32(white[sl].T), fb=f32(black[sl].T), wT=wT, ftb=ftb,
            stm=f32(stm[sl]), l1w=l1wT, l1b=l1b, l2w=l2wT, l2b=l2b,
            l3w=l3wT, l3b=l3b))
    return in_maps


def _assemble(results):
    sig = np.concatenate([results[c]["out"][0] for c in range(NCORES)])
    raw = np.concatenate([results[c]["out"][1] for c in range(NCORES)])
    return (sig.reshape(B, 1).astype(np.float32),
            raw.reshape(B, 1).astype(np.float32))


def kernel(**inputs):
    nc = _get_nc()
    in_maps = _prep_in_maps(**inputs)
    res = run_bass_kernel_spmd(nc, in_maps, core_ids=list(range(NCORES)))
    return _assemble(res.results)
